# revision 51
# baseline (speedup 1.0000x reference)
"""Distributed Bass kernel for a dense-transformer attention layer on 8 TRN2 cores.

v2 strategy (tensor-parallel over heads, all-bf16 matmul operands, zero DRAM
round-trips for Q/K/V):
  - Host: xT bf16 [H, T]; per-core column-shards of Wq/Wk (rows permuted per
    head so rotary pairs land in partition halves) and Wv, all bf16; full
    Wd.T bf16; bf16 cos/sin tables [hd, T] (sin pre-signed for the half-swap
    formulation); one [128,128] triangle mask tile.
  - Device, per core (SPMD, 2 heads each):
      A) QKV projections streamed over 8 x-slabs of 512 tokens; Q_T/K_T are
         rotated (RoPE) during PSUM eviction: Act copy -> SBUF, partition-half
         swap by local SBUF DMA, two muls (DVE+Pool) and an add (DVE); the
         rotated Q_T/K_T [hd, hpc, T] and V [tok, f] stay RESIDENT in SBUF.
      B) Attention per (batch, head): scores computed transposed
         S_T[kpos, q] = K_tile(lhsT) @ Q block with causal block skipping and
         diagonal trimming (partial blocks start at their first live query
         column; only a [128,128] triangle mask multiply remains), exp without
         max-subtraction (tiny logits), denominator by ones-matmul, context
         ctx_T[d, q] += V_tile(lhsT) @ P_T.  Normalized bf16 ctx is written to
         a per-(head,batch) DRAM buffer and AllToAll'd immediately (4 small
         512KB collectives, pipelined with remaining attention).
      C) Output projection on this core's 2x256-token blocks: 16-way PSUM
         accumulation over (src core, head) with resident bf16 Wd.T;
         batch-0 blocks run while the last collective is still in flight.
  - Host: reassemble the 8 interleaved 256-token blocks -> [B, S, H].
"""

import os
import sys
import math
from dataclasses import dataclass, field

import numpy as np

sys.path.insert(0, "/opt/trn_rl_repo")

# ---------------------------------------------------------------- problem dims
B, S, H, NH = 2, 2048, 2048, 16
HD = H // NH  # 128
NCORES = 8
ROPE_BASE = 10000.0
SCALE = 1.0 / math.sqrt(HD)

KB = 128    # key-block (kpos per score tile)
QB = 512    # query-block (free dim of score tiles)
TBLK = 256  # per-batch token block owned by each core

LAST_EXEC_NS = None


@dataclass
class Config:
    b: int = B
    s: int = S
    h: int = H
    nh: int = NH
    ncores: int = NCORES
    qb: int = QB
    qbp: int = 512  # token-chunk width for the projection stage
    no_cc: bool = False  # replace collectives with local DMA (for TimelineSim)
    stages: str = "abc"  # subset of stages to build (timing bisection)
    cc2: bool = False    # 2 per-batch collectives instead of 4 per-(head,batch)
    exp_copy: bool = False  # timing bisect: replace Exp with Copy (wrong math)
    no_dn: bool = False     # timing bisect: skip denominator path (wrong math)
    # blocks[jq] = list of (k, qoff, mask_idx|None)
    blocks: list = field(default_factory=list)
    n_bias: int = 0

    @property
    def hd(self):
        return self.h // self.nh

    @property
    def t(self):
        return self.b * self.s

    @property
    def hpc(self):  # heads per core
        return self.nh // self.ncores

    @property
    def f(self):  # features per core
        return self.hpc * self.hd

    @property
    def nb(self):  # tokens per core output block
        return self.t // self.ncores


def classify_blocks(mask2d: np.ndarray, qb: int, kb: int):
    """mask2d: [S, S] bool, True = masked out.

    Returns (blocks, mask_tiles): blocks[jq] is a list of (k, qoff, mask_idx)
    where queries < qoff are fully masked for key-block k, the [qoff, qoff+128)
    query columns carry the (transposed) mask tile (None if fully live), and
    queries >= qoff+128 are fully live.  mask_tiles[i] is [kb, 128] f32 0/1.
    """
    s = mask2d.shape[0]
    tiles = []
    keys = {}
    bmap = []
    for jq in range(s // qb):
        row = []
        for k in range(s // kb):
            sub = mask2d[jq * qb:(jq + 1) * qb, k * kb:(k + 1) * kb]
            if sub.all():
                continue
            col_any_keep = ~sub.all(axis=1)
            first = int(np.argmax(col_any_keep))
            qoff = (first // 128) * 128
            assert qoff == 0 or sub[:qoff].all(), "unsupported mask shape"
            mblk = sub[qoff:qoff + 128]
            rest = sub[qoff + 128:]
            assert not rest.any(), "unsupported mask shape"
            if not mblk.any():
                row.append((k, qoff, None))
                continue
            # bias tile: 1.0 where masked (multiplied by -10000 via eye-matmul)
            tile = np.where(mblk.T, np.float32(1.0), np.float32(0.0))
            key = tile.tobytes()
            if key not in keys:
                keys[key] = len(tiles)
                tiles.append(np.ascontiguousarray(tile, np.float32))
            row.append((k, qoff, keys[key]))
        assert row, "fully-masked query block"
        assert row[0][1] == 0, "first block must cover the full query range"
        bmap.append(row)
    return bmap, tiles


# ------------------------------------------------------------------ host sharding
def prepare(hidden_states, Wq, Wk, Wv, Wd, attention_mask, position_ids, cfg):
    import ml_dtypes

    bf16 = ml_dtypes.bfloat16
    s, h, nh, hd = cfg.s, cfg.h, cfg.nh, cfg.hd
    t = cfg.t

    x = np.asarray(hidden_states, np.float32).reshape(t, h)
    xT = np.ascontiguousarray(x.T.astype(bf16))  # [H, T] bf16

    # per-head pair permutation: [0,2,...,hd-2, 1,3,...,hd-1]
    pp = np.concatenate([np.arange(0, hd, 2), np.arange(1, hd, 2)])
    perm = np.concatenate([hh * hd + pp for hh in range(nh)])

    WqP = np.asarray(Wq, np.float32)[perm]
    WkP = np.asarray(Wk, np.float32)[perm]
    Wv_ = np.asarray(Wv, np.float32)
    WdT = np.ascontiguousarray(np.asarray(Wd, np.float32).T.astype(bf16))  # [H, H]

    inv_freq = (1.0 / (ROPE_BASE ** (np.arange(0, hd, 2, dtype=np.float32) / np.float32(hd)))).astype(np.float32)
    pos = np.asarray(position_ids).astype(np.float32).reshape(t)  # [T]
    ang = pos[None, :] * inv_freq[:, None]  # [hd/2, T]
    cos = np.cos(ang).astype(np.float32)
    sin = np.sin(ang).astype(np.float32)
    cosT = np.ascontiguousarray(np.concatenate([cos, cos], axis=0).astype(bf16))
    sinT = np.ascontiguousarray(np.concatenate([-sin, sin], axis=0).astype(bf16))

    mask2d = np.asarray(attention_mask).reshape(s, s).astype(bool)
    cfg.blocks, mask_tiles = classify_blocks(mask2d, cfg.qb, KB)
    cfg.n_bias = len(mask_tiles)
    maskb = None
    if cfg.n_bias:
        maskb = np.ascontiguousarray(np.stack(mask_tiles, axis=0).astype(bf16))

    in_maps = []
    f = cfg.f
    for c in range(cfg.ncores):
        m = {
            "xT": xT,
            "wqT": np.ascontiguousarray(WqP[c * f:(c + 1) * f].T.astype(bf16)),  # [H, F]
            "wkT": np.ascontiguousarray(WkP[c * f:(c + 1) * f].T.astype(bf16)),
            "wvT": np.ascontiguousarray(Wv_[c * f:(c + 1) * f].T.astype(bf16)),
            "wdT": WdT,
            "cosT": cosT,
            "sinT": sinT,
            "ones_bf": np.ones((128, 128), bf16),
            "negeye": (np.eye(128, dtype=np.float32) * np.float32(-10000.0)).astype(bf16),
        }
        if cfg.n_bias:
            m["maskb"] = maskb
        in_maps.append(m)
    return in_maps


# ------------------------------------------------------------------ graph builder
def build_graph(cfg, repeat=1):
    import concourse.tile as tile
    from concourse import bacc, mybir

    dt = mybir.dt
    bf = dt.bfloat16
    f32 = dt.float32
    f32r = dt.float32r

    b, s, h = cfg.b, cfg.s, cfg.h
    t, f, hd, hpc = cfg.t, cfg.f, cfg.hd, cfg.hpc
    nb, qb = cfg.nb, cfg.qb
    nkt = h // 128           # k-tiles over hidden dim
    npc = t // cfg.qbp       # token chunks in projection stage
    assert hd == 128 and cfg.qbp == 512 and qb == 512

    nc = bacc.Bacc(None, target_bir_lowering=False)

    xT = nc.declare_dram_parameter("xT", [h, t], bf, isOutput=False)
    wqT = nc.declare_dram_parameter("wqT", [h, f], bf, isOutput=False)
    wkT = nc.declare_dram_parameter("wkT", [h, f], bf, isOutput=False)
    wvT = nc.declare_dram_parameter("wvT", [h, f], bf, isOutput=False)
    wdT = nc.declare_dram_parameter("wdT", [h, h], bf, isOutput=False)
    cosT = nc.declare_dram_parameter("cosT", [hd, t], bf, isOutput=False)
    sinT = nc.declare_dram_parameter("sinT", [hd, t], bf, isOutput=False)
    ones_bf_d = nc.declare_dram_parameter("ones_bf", [128, 128], bf, isOutput=False)
    negeye_d = nc.declare_dram_parameter("negeye", [128, 128], bf, isOutput=False)
    if cfg.n_bias:
        maskb = nc.declare_dram_parameter("maskb", [cfg.n_bias, KB, 128], bf, isOutput=False)
    out = nc.declare_dram_parameter("out", [nb, h], f32, isOutput=True)

    def mm(o, lhsT, rhs, start, stop, skip_group_check=False):
        nc.tensor.matmul(o, lhsT, rhs, start=start, stop=stop,
                         skip_group_check=skip_group_check)

    xT3 = xT.ap().rearrange("(k p) t -> p k t", p=128)       # [128, nkt, T]
    wdT3 = wdT.ap().rearrange("(k p) o -> p k o", p=128)     # [128, nkt, H]
    out3 = out.ap().rearrange("(r p) o -> p r o", p=128)     # [128, 4, H]

    with tile.TileContext(nc) as tc:
        with (
            tc.tile_pool(name="persist", bufs=1) as persist,
            tc.tile_pool(name="dram", bufs=1, space="DRAM") as dram,
            tc.tile_pool(name="psum", bufs=1, space="PSUM") as psum,
        ):
            # Startup-latency critical path: only wq + the first half x-slab
            # gate the first matmul; everything else is issued after them.
            ones_bf_sb = persist.tile([128, 128], bf, name="ones_bf_sb")
            negeye_sb = persist.tile([128, 128], bf, name="negeye_sb")
            mask_sb = [persist.tile([KB, 128], bf, name=f"mask_sb{i}")
                       for i in range(cfg.n_bias)]
            cos_sb = persist.tile([hd, t], bf, name="cos_sb")
            sin_sb = persist.tile([hd, t], bf, name="sin_sb")
            wq_sb = persist.tile([128, nkt, f], bf, name="wq_sb")
            wk_sb = persist.tile([128, nkt, f], bf, name="wk_sb")
            wv_sb = persist.tile([128, nkt, f], bf, name="wv_sb")
            wq3 = wqT.ap().rearrange("(k p) f -> p k f", p=128)
            wk3 = wkT.ap().rearrange("(k p) f -> p k f", p=128)
            wv3 = wvT.ap().rearrange("(k p) f -> p k f", p=128)
            nc.sync.dma_start(out=wq_sb[:], in_=wq3[:, :, :])
            wd_sb = persist.tile([128, nkt, h], bf, name="wd_sb")

            for _rep in range(repeat):
                with (
                    tc.tile_pool(name="ab", bufs=1) as ab,
                    tc.tile_pool(name="c_io", bufs=1) as c_io,
                ):
                    q_sb = ab.tile([128, hpc, t], bf, name="q_sb", tag="q_sb")
                    k_sb = ab.tile([128, hpc, t], bf, name="k_sb", tag="k_sb")
                    v_sb = ab.tile([128, t // 128, f], bf, name="v_sb", tag="v_sb")

                    # ---------------- stage A: QKV projections + RoPE ----------------
                    with (
                        tc.tile_pool(name="a_x", bufs=2) as a_x,
                        tc.tile_pool(name="a_t", bufs=2) as a_t,
                    ):
                        nk2 = nkt // 2
                        for c in range(npc):
                            cs = slice(c * 512, (c + 1) * 512)
                            xa = a_x.tile([128, nk2, 512], bf, name="xa", tag="xa")
                            xb = a_x.tile([128, nk2, 512], bf, name="xb", tag="xb")
                            nc.sync.dma_start(out=xa[:], in_=xT3[:, :nk2, cs])
                            if _rep == 0 and c == 0:
                                nc.sync.dma_start(out=wk_sb[:], in_=wk3[:, :, :])
                                nc.sync.dma_start(out=wv_sb[:], in_=wv3[:, :, :])
                            nc.sync.dma_start(out=xb[:], in_=xT3[:, nk2:, cs])
                            if _rep == 0 and c == 0:
                                nc.sync.dma_start(out=cos_sb[:], in_=cosT[:, :])
                                nc.sync.dma_start(out=sin_sb[:], in_=sinT[:, :])
                            if _rep == 0 and c == 1:
                                nc.sync.dma_start(out=ones_bf_sb[:], in_=ones_bf_d[:, :])
                                nc.sync.dma_start(out=negeye_sb[:], in_=negeye_d[:, :])
                                for i in range(cfg.n_bias):
                                    nc.sync.dma_start(out=mask_sb[i][:], in_=maskb[i, :, :])

                            def xs(k):
                                return xa[:, k, :] if k < nk2 else xb[:, k - nk2, :]

                            for w_sb, dst in ((wq_sb, q_sb), (wk_sb, k_sb)):
                                for ft in range(hpc):
                                    ps = psum.tile([128, 512], f32,
                                                   name="proj_ps", tag="st_ps", bufs=4)
                                    for k in range(nkt):
                                        mm(ps[:], w_sb[:, k, ft * 128:(ft + 1) * 128],
                                           xs(k), start=(k == 0), stop=(k == nkt - 1))
                                    tq = a_t.tile([128, 512], bf, name="tq", tag="tq")
                                    nc.scalar.copy(tq[:], ps[:])
                                    tsw = a_t.tile([128, 512], bf, name="tsw", tag="tsw")
                                    nc.sync.dma_start(out=tsw[0:64, :], in_=tq[64:128, :])
                                    nc.sync.dma_start(out=tsw[64:128, :], in_=tq[0:64, :])
                                    nc.vector.tensor_mul(tq[:], tq[:], cos_sb[:, cs])
                                    nc.gpsimd.tensor_mul(tsw[:], tsw[:], sin_sb[:, cs])
                                    nc.vector.tensor_add(dst[:, ft, cs], tq[:], tsw[:])

                            for ts in range(4):
                                ps = psum.tile([128, f], f32,
                                               name="v_ps", tag="st_ps", bufs=4)
                                for k in range(nkt):
                                    mm(ps[:], xs(k)[:, ts * 128:(ts + 1) * 128],
                                       wv_sb[:, k, :], start=(k == 0), stop=(k == nkt - 1))
                                nc.scalar.copy(v_sb[:, c * 4 + ts, :], ps[:])

                            if _rep == 0 and 4 <= c <= 7:
                                kk = (c - 4) * 4
                                nc.sync.dma_start(out=wd_sb[:, kk:kk + 4, :],
                                                  in_=wdT3[:, kk:kk + 4, :])

                    # ---------------- stage B: attention ----------------
                    if "b" not in cfg.stages:
                        continue
                    a2a_in = {}
                    a2a_out = {}
                    if cfg.cc2:
                        for bb in range(b):
                            a2a_in[bb] = dram.tile(
                                [cfg.ncores, hpc, hd, TBLK], bf,
                                name=f"a2a_in{bb}", tag=f"a2a_in{bb}")
                            a2a_out[bb] = dram.tile(
                                [cfg.ncores, hpc, hd, TBLK], bf,
                                name=f"a2a_out{bb}", tag=f"a2a_out{bb}")
                    else:
                        for hi in range(hpc):
                            for bb in range(b):
                                a2a_in[(hi, bb)] = dram.tile(
                                    [cfg.ncores, hd, TBLK], bf,
                                    name=f"a2a_in{hi}{bb}", tag=f"a2a_in{hi}{bb}")
                                a2a_out[(hi, bb)] = dram.tile(
                                    [cfg.ncores, hd, TBLK], bf,
                                    name=f"a2a_out{hi}{bb}", tag=f"a2a_out{hi}{bb}")

                    njq = s // qb
                    cf = {}
                    cf_loads = []  # deferred cf DMA closures (SP queue)
                    with (
                        tc.tile_pool(name="b_p", bufs=8) as b_p,
                        tc.tile_pool(name="b_acc", bufs=2) as b_acc,
                        tc.tile_pool(name="b_d", bufs=2) as b_d,
                    ):
                        for bb in range(b):
                            base = bb * s
                            for hi in range(hpc):
                                if len(cf_loads) > 1:
                                    cf_loads.pop(0)()
                                ccat = b_acc.tile([hd, 2 * njq, TBLK], bf,
                                                  name="ccat", tag="ccat")

                                # Flattened, software-pipelined (depth 3) block
                                # stream across all jq; each jq's normalize
                                # sequence is deferred so its PE ops land well
                                # after the DVE reciprocal they depend on.
                                pts = []       # (pt, qoff, k, dn, ctx_ps, i, nblk)
                                deferred = []  # normalize closures

                                def drain_one():
                                    pt, qoff, k, dn, ctx_ps, i, nblk = pts.pop(0)
                                    if not cfg.no_dn:
                                        mm(dn[:, qoff:], ones_bf_sb[:, 0:1], pt[:, qoff:],
                                           start=(i == 0), stop=(i == nblk - 1),
                                           skip_group_check=True)
                                    mm(ctx_ps[:, qoff:],
                                       v_sb[:, bb * 16 + k, hi * hd:(hi + 1) * hd],
                                       pt[:, qoff:],
                                       start=(i == 0), stop=(i == nblk - 1),
                                       skip_group_check=True)
                                    if i == nblk - 1:
                                        deferred.append(make_norm(dn, ctx_ps))

                                def make_norm(dn, ctx_ps, jq_=None):
                                    jq = make_norm.jq = getattr(make_norm, "jq", -1) + 1
                                    jq = jq % njq

                                    def norm():
                                        if cfg.no_dn:
                                            nc.vector.tensor_copy(
                                                ccat[:, jq * 2:(jq + 1) * 2, :], ctx_ps[:])
                                            return
                                        rsm = b_d.tile([1, qb], f32, name="rsm", tag="rsm")
                                        nc.vector.reciprocal(rsm[:], dn[:])
                                        rsb = b_d.tile([1, qb], bf, name="rsb", tag="rsb")
                                        nc.vector.tensor_copy(rsb[:], rsm[:])
                                        bps = psum.tile([128, qb], f32,
                                                        name="bps", tag="dn_ps", bufs=2)
                                        mm(bps[:], ones_bf_sb[0:1, :], rsb[:],
                                           start=True, stop=True)
                                        rb = b_d.tile([128, qb], f32, name="rb", tag="rb")
                                        nc.vector.tensor_copy(rb[:], bps[:])
                                        nc.vector.tensor_mul(
                                            ccat[:, jq * 2:(jq + 1) * 2, :],
                                            ctx_ps[:], rb[:])
                                    return norm

                                for jq in range(njq):
                                    blocks = cfg.blocks[jq]
                                    nblk = len(blocks)
                                    q0 = base + jq * qb
                                    ctx_ps = psum.tile([hd, qb], f32,
                                                       name="ctx_ps", tag="ctx_ps", bufs=2)
                                    dn = psum.tile([1, qb], f32,
                                                   name="dn_ps", tag="dn_ps", bufs=2)
                                    for i, (k, qoff, mi) in enumerate(blocks):
                                        st = psum.tile([KB, qb], f32,
                                                       name="st_ps", tag="st_ps", bufs=4)
                                        mm(st[:, qoff:], k_sb[:, hi, base + k * KB:base + (k + 1) * KB],
                                           q_sb[:, hi, q0 + qoff:q0 + qb],
                                           start=True, stop=(mi is None),
                                           skip_group_check=True)
                                        if mi is not None:
                                            # -10000 bias on masked entries of the
                                            # diagonal 128-col strip, on the PE
                                            mm(st[:, qoff:qoff + 128], negeye_sb[:, :],
                                               mask_sb[mi][:], start=False, stop=True,
                                               skip_group_check=True)
                                        if deferred:
                                            deferred.pop(0)()
                                        pt = b_p.tile([KB, qb], bf, name="pt", tag="pt")
                                        nc.scalar.activation(
                                            pt[:, qoff:], st[:, qoff:],
                                            mybir.ActivationFunctionType.Copy
                                            if cfg.exp_copy else
                                            mybir.ActivationFunctionType.Exp,
                                            scale=float(SCALE),
                                        )
                                        pts.append((pt, qoff, k, dn, ctx_ps, i, nblk))
                                        if len(pts) > 3:
                                            drain_one()
                                while pts:
                                    drain_one()
                                while deferred:
                                    deferred.pop(0)()
                                if cfg.cc2:
                                    nc.sync.dma_start(
                                        out=a2a_in[bb].rearrange(
                                            "c h p n -> p c h n")[:, :, hi, :],
                                        in_=ccat[:],
                                    )
                                    if hi != hpc - 1:
                                        continue
                                    a_in, a_out = a2a_in[bb], a2a_out[bb]
                                    cf_shape = [128, cfg.ncores, hpc, TBLK]
                                    cf_key, cf_nm = bb, f"cfb{bb}"

                                    def cf_src(a_out=a_out):
                                        return a_out.rearrange("c h p n -> p c h n")
                                else:
                                    nc.sync.dma_start(
                                        out=a2a_in[(hi, bb)].rearrange("c p n -> p c n"),
                                        in_=ccat[:],
                                    )
                                    a_in, a_out = a2a_in[(hi, bb)], a2a_out[(hi, bb)]
                                    cf_shape = [128, cfg.ncores, TBLK]
                                    cf_key, cf_nm = (hi, bb), f"cf{hi}{bb}"

                                    def cf_src(a_out=a_out):
                                        return a_out.rearrange("c p n -> p c n")
                                if cfg.no_cc:
                                    nc.sync.dma_start(out=a_out[:], in_=a_in[:])
                                else:
                                    nc.gpsimd.collective_compute(
                                        "AllToAll",
                                        mybir.AluOpType.bypass,
                                        replica_groups=[list(range(cfg.ncores))],
                                        ins=[a_in.opt()],
                                        outs=[a_out.opt()],
                                    )
                                tl_ = c_io.tile(cf_shape, bf, name=cf_nm, tag=cf_nm)
                                cf[cf_key] = tl_

                                def load_cf(tl_=tl_, cf_src=cf_src):
                                    nc.sync.dma_start(out=tl_[:], in_=cf_src())
                                cf_loads.append(load_cf)
                        while cf_loads:
                            cf_loads.pop(0)()

                    # ---------------- stage C: output projection ----------------
                    if "c" not in cfg.stages:
                        continue
                    with (
                        tc.tile_pool(name="c_o", bufs=2) as c_o,
                    ):
                        for bb in range(b):
                            for n in range(h // 512):
                                ocat = c_o.tile([128, 2, 512], f32, name="ocat", tag="ocat")
                                for ts in range(2):
                                    ps = psum.tile([128, 512], f32,
                                                   name="o_ps", tag="st_ps", bufs=4)
                                    for cc in range(cfg.ncores):
                                        for hi in range(hpc):
                                            if cfg.cc2:
                                                lhsT = cf[bb][:, cc, hi, ts * 128:(ts + 1) * 128]
                                            else:
                                                lhsT = cf[(hi, bb)][:, cc, ts * 128:(ts + 1) * 128]
                                            mm(ps[:], lhsT,
                                               wd_sb[:, cc * hpc + hi, n * 512:(n + 1) * 512],
                                               start=(cc == 0 and hi == 0),
                                               stop=(cc == cfg.ncores - 1 and hi == hpc - 1))
                                    nc.scalar.copy(ocat[:, ts, :], ps[:])
                                nc.scalar.dma_start(
                                    out=out3[:, bb * 2:bb * 2 + 2, n * 512:(n + 1) * 512],
                                    in_=ocat[:],
                                )
    nc.compile()
    return nc


# ------------------------------------------------------------------ executor
def _prepare_exec_full(nc, in_maps, n_cores):
    """Build the sharded jit callable + device-resident args for nc."""
    import jax
    from jax.experimental.shard_map import shard_map
    from jax.sharding import Mesh, NamedSharding, PartitionSpec

    from concourse import bass2jax, mybir

    bass2jax.install_neuronx_cc_hook()
    assert nc.dbg_addr is None or not nc.dbg_callbacks

    partition_name = nc.partition_id_tensor.name if nc.partition_id_tensor else None
    in_names, out_names, out_avals, zero_outs = [], [], [], []
    for alloc in nc.m.functions[0].allocations:
        if not isinstance(alloc, mybir.MemoryLocationSet):
            continue
        name = alloc.memorylocations[0].name
        if alloc.kind == "ExternalInput":
            if name != partition_name and name != (nc.dbg_addr.name if nc.dbg_addr else None):
                in_names.append(name)
        elif alloc.kind == "ExternalOutput":
            shape = tuple(alloc.tensor_shape)
            dtype = mybir.dt.np(alloc.dtype)
            out_avals.append(jax.core.ShapedArray(shape, dtype))
            out_names.append(name)
            zero_outs.append(np.zeros(shape, dtype))
    n_params = len(in_names)
    all_in_names = list(in_names) + list(out_names)
    if nc.dbg_addr is not None:
        in_maps = [
            {**m, nc.dbg_addr.name: np.zeros((1, 2), np.uint32)} for m in in_maps
        ]
        all_in_names.append(nc.dbg_addr.name)
        n_dbg = 1
    else:
        n_dbg = 0
    if partition_name is not None:
        all_in_names.append(partition_name)

    def _body(*args):
        operands = list(args)
        if partition_name is not None:
            operands.append(bass2jax.partition_id_tensor())
        outs = bass2jax._bass_exec_p.bind(
            *operands,
            out_avals=tuple(out_avals),
            in_names=tuple(all_in_names),
            out_names=tuple(out_names),
            lowering_input_output_aliases=(),
            sim_require_finite=True,
            sim_require_nnan=True,
            nc=nc,
        )
        return tuple(outs)

    devices = jax.devices()[:n_cores]
    assert len(devices) == n_cores
    mesh = Mesh(np.asarray(devices), ("core",))
    n_ops = n_params + len(out_names) + n_dbg
    sharded = jax.jit(
        shard_map(
            _body,
            mesh=mesh,
            in_specs=(PartitionSpec("core"),) * n_ops,
            out_specs=(PartitionSpec("core"),) * len(out_names),
            check_rep=False,
        ),
        keep_unused=True,
    )
    sh = NamedSharding(mesh, PartitionSpec("core"))
    dev_args = []
    for i, name in enumerate(all_in_names[:n_params]):
        cat = np.concatenate([np.asarray(m[name]) for m in in_maps], axis=0)
        dev_args.append(jax.device_put(cat, sh))
    for z in zero_outs:
        cat = np.zeros((n_cores * z.shape[0], *z.shape[1:]), z.dtype)
        dev_args.append(jax.device_put(cat, sh))
    if n_dbg:
        name = nc.dbg_addr.name
        cat = np.concatenate([np.asarray(m[name]) for m in in_maps], axis=0)
        dev_args.append(jax.device_put(cat, sh))
    return sharded, dev_args, out_names, out_avals


def _prepare_exec(nc, in_maps, n_cores):
    fn, args, _, _ = _prepare_exec_full(nc, in_maps, n_cores)
    return fn, args


def _execute(nc, in_maps, n_cores, n_timed=0):
    import time as _time

    import jax

    sharded, dev_args, out_names, out_avals = _prepare_exec_full(nc, in_maps, n_cores)
    out_arrs = sharded(*dev_args)
    jax.block_until_ready(out_arrs)

    timed = None
    if n_timed > 0:
        times = []
        for _ in range(n_timed):
            t0 = _time.perf_counter()
            r = sharded(*dev_args)
            jax.block_until_ready(r)
            times.append(_time.perf_counter() - t0)
        timed = int(min(times) * 1e9)

    results = [
        {
            name: np.asarray(out_arrs[i]).reshape(n_cores, *out_avals[i].shape)[c]
            for i, name in enumerate(out_names)
        }
        for c in range(n_cores)
    ]
    return results, timed


# ------------------------------------------------------------------ entry point
def kernel(hidden_states, Wq, Wk, Wv, Wd, attention_mask, position_ids):
    global LAST_EXEC_NS
    cfg = Config()
    in_maps = prepare(hidden_states, Wq, Wk, Wv, Wd, attention_mask, position_ids, cfg)
    nc = build_graph(cfg)

    n_timed = int(os.environ.get("BASS_KERNEL_TIME", "0"))
    results, timed = _execute(nc, in_maps, cfg.ncores, n_timed=n_timed)
    LAST_EXEC_NS = timed
    full = np.zeros((B, S, H), np.float32)
    for c in range(cfg.ncores):
        o = np.asarray(results[c]["out"], np.float32)  # [512, H]
        for bb in range(B):
            full[bb, c * TBLK:(c + 1) * TBLK] = o[bb * TBLK:(bb + 1) * TBLK]
    return full
